# revision 9
# baseline (speedup 1.0000x reference)
"""Trainium2 Bass kernel for fake-quant (W8A8) linear: y = fq_tok(x) @ fq_ch(w).T + b.

Full shapes: x [4, 2048, 4096] f32, w [4096, 4096] f32, b [4096] f32.
Sharding over 8 cores: 2 token groups x 4 out-channel groups.
Per core: x_sh [4096, 4096], w_sh [1024, 4096], b_sh [1024] -> y_sh [4096, 1024].

Quantized values are integers in [-127, 127], exactly representable in bf16,
so the matmul runs on the PE array in bf16 (full rate) with fp32 PSUM
accumulation - numerically equivalent to the fp32 reference einsum on the
dequantized values.  Scales are applied in the fp32 epilogue.

Design (v5):
- All 128x128 transposes run on the DMA xbar (`dma_start(transpose=True)`),
  one 1MB transpose per token tile / w block.  The PE does nothing but the
  2048 N=512 bf16 matmuls (216 ns roofline each).
- HWDGE ring discipline (hard-won): the transpose ucode op BLOCKS its ring
  ~5us AND the scheduler serializes any same-ring DMA against an in-flight
  transpose, so the Sync (SP) ring carries ONLY transposes; every load /
  store / broadcast is issued from the ACT ring (regular DMAs there are
  0.6us async issues).  Transposes on both rings at once crash the device
  (probed: NRT_EXEC_UNIT_UNRECOVERABLE) - never split them.
- Steady state is an explicit software pipeline: iteration tt issues
  y-stores(tt-1), load(tt+2), quant(tt+2) [DVE amax + ACT rounds],
  transpose(tt+1), matmul(tt), epilogue(tt).  The skew keeps every queue
  wait near zero; without it the tile period balloons past 17us.
- Startup: qwT is split into two 512-channel halves; x0 is quantized first
  so its transpose leads the Sync ring; the PE runs cb0 groups of tiles
  0-2 as soon as w blocks 0-3 are transposed, cb1 once w4-7 land.  w4-7
  round-pass-1 runs on DVE to unclog ACT's weight-phase queue.

Rounding: round-half-to-even via the fp32 magic-constant trick
(v + 1.5*2^23 rounds mantissa to integer; subtract again afterwards),
matching jnp.round.  Clipping to [-128, 127] is a no-op by construction
(|x|/s <= 127 when s = amax/127) so it is skipped.
"""

from contextlib import ExitStack

import numpy as np

import concourse.bass as bass
import concourse.mybir as mybir
import concourse.tile as tile
from concourse import bacc

P = 128
MAGIC = 12582912.0  # 1.5 * 2**23
QMAX = 127.0
EPS = 1e-8

# full problem shapes (hardcoded per harness contract)
B, S, D_IN, D_OUT = 4, 2048, 4096, 4096
TOK = B * S  # 8192
TOK_GROUPS = 2
CH_GROUPS = 4
T_SH = TOK // TOK_GROUPS  # 4096 tokens per core
O_SH = D_OUT // CH_GROUPS  # 1024 channels per core


def build_nc(T, K, O, nch=512):
    """Build the per-core Bass program: x[T,K], w[O,K], b[O] -> y[T,O]."""
    f32 = mybir.dt.float32
    bf16 = mybir.dt.bfloat16
    Copy = mybir.ActivationFunctionType.Copy
    Alu = mybir.AluOpType
    AxX = mybir.AxisListType.X

    assert T % P == 0 and K % P == 0 and O % P == 0
    TT, KB, WT = T // P, K // P, O // P
    NCH = min(nch, O)
    CB = O // NCH
    assert CB == 2 and WT == 8, "startup interleave assumes 2 halves x 4 blocks"
    WPH = WT // CB  # w blocks per qwT half
    NPRE = 3  # x tiles quantized+transposed during the w phase

    nc = bacc.Bacc("TRN2", target_bir_lowering=False, debug=False)
    x_ap = nc.dram_tensor("x", [T, K], f32, kind="ExternalInput").ap()
    w_ap = nc.dram_tensor("w", [O, K], f32, kind="ExternalInput").ap()
    b_ap = nc.dram_tensor("b", [O], f32, kind="ExternalInput").ap()
    y_ap = nc.dram_tensor("y", [T, O], f32, kind="ExternalOutput").ap()

    with tile.TileContext(nc) as tc, ExitStack() as ctx:
        singles = ctx.enter_context(tc.tile_pool(name="singles", bufs=1))
        bigf32 = ctx.enter_context(tc.tile_pool(name="bigf32", bufs=3))
        rnd = ctx.enter_context(tc.tile_pool(name="rnd", bufs=1))
        qpool = ctx.enter_context(tc.tile_pool(name="qpool", bufs=3))
        qtpool = ctx.enter_context(tc.tile_pool(name="qtpool", bufs=3))
        stats = ctx.enter_context(tc.tile_pool(name="stats", bufs=8))
        sxpool = ctx.enter_context(tc.tile_pool(name="sxpool", bufs=5))
        opool = ctx.enter_context(tc.tile_pool(name="opool", bufs=6))
        psum_pool = ctx.enter_context(tc.tile_pool(name="psum", bufs=6, space="PSUM"))
        dram = ctx.enter_context(tc.tile_pool(name="dram", bufs=1, space="DRAM"))

        # resident: transposed quantized weights (two 512-ch halves so cb0
        # matmuls only depend on w blocks 0-3) + broadcast scale/bias rows
        qwT = [singles.tile([P, KB, NCH], bf16, name=f"qwT{h}") for h in range(CB)]
        sw_b = singles.tile([P, O], f32)
        bb_b = singles.tile([P, O], f32)
        sw_dram = dram.tile([O, 1], f32)

        def quantize(src_t, q_t, s_t, dve_round=False):
            # per-row amax -> scale (s_t), then round src*(1/s) to q_t (bf16)
            amax = stats.tile([P, 1], f32, tag="st", name="amax")
            nc.vector.reduce_max(
                out=amax, in_=src_t, axis=AxX, apply_absolute_value=True
            )
            nc.vector.tensor_scalar(
                out=s_t, in0=amax, scalar1=1.0 / QMAX, scalar2=EPS,
                op0=Alu.mult, op1=Alu.max,
            )
            r_t = stats.tile([P, 1], f32, tag="st", name="recip")
            nc.vector.reciprocal(out=r_t, in_=s_t)
            t_t = rnd.tile([P, K], f32, tag="rnd", name="t_round")
            if dve_round:
                # pass 1 on DVE (weight-phase load balancing)
                nc.vector.tensor_scalar(
                    out=t_t, in0=src_t, scalar1=r_t[:, 0:1], scalar2=MAGIC,
                    op0=Alu.mult, op1=Alu.add,
                )
            else:
                # pass 1 on ACT (scale is a per-partition pointer operand; the
                # Bacc event-semaphore pass legalizes its single-wait limit)
                nc.scalar.activation(
                    out=t_t, in_=src_t, func=Copy, bias=MAGIC, scale=r_t[:, 0:1]
                )
            nc.scalar.activation(out=q_t, in_=t_t, func=Copy, bias=-MAGIC, scale=1.0)

        # ---- stages.  Loads/stores: ACT ring.  Transposes: Sync ring. ----
        def load_w(wt):
            w_t = bigf32.tile([P, K], f32, tag="big", name=f"w_{wt}")
            nc.scalar.dma_start(out=w_t, in_=w_ap[wt * P : (wt + 1) * P, :])
            return w_t

        def quant_w(wt, w_t):
            sw = stats.tile([P, 1], f32, tag="st", name=f"sw_{wt}")
            qw = qpool.tile([P, K], bf16, tag="q", name=f"qw_{wt}")
            quantize(w_t, qw, sw, dve_round=(wt >= WPH))
            nc.scalar.dma_start(out=sw_dram[wt * P : (wt + 1) * P, :], in_=sw)
            return qw

        def transpose_w(wt, qw):
            h, c = divmod(wt, WPH)
            nc.sync.dma_start(
                out=qwT[h][:, :, c * P : (c + 1) * P], in_=qw, transpose=True
            )

        def load_x(tt):
            x_t = bigf32.tile([P, K], f32, tag="big", name=f"x_{tt}")
            nc.scalar.dma_start(out=x_t, in_=x_ap[tt * P : (tt + 1) * P, :])
            return x_t

        def quant_x(tt, x_t):
            sx = sxpool.tile([P, 1], f32, tag="sx", name=f"sx_{tt}")
            qx = qpool.tile([P, K], bf16, tag="q", name=f"qx_{tt}")
            quantize(x_t, qx, sx)
            return sx, qx

        def transpose_x(tt, qx):
            qxT = qtpool.tile([P, KB, P], bf16)  # qxT[f, k, t] = qx[t, k*128+f]
            nc.sync.dma_start(out=qxT, in_=qx, transpose=True)
            return qxT

        def mm_group(tt, cb, qxT):
            ps = psum_pool.tile([P, NCH], f32, tag="psum", name=f"ps_{tt}_{cb}")
            for k in range(KB):
                nc.tensor.matmul(
                    ps,
                    qxT[:, k, :],
                    qwT[cb][:, k, :],
                    start=(k == 0),
                    stop=(k == KB - 1),
                )
            return ps

        def epilogue(tt, cb, sx, ps):
            # y = (psum * sx) * sw + b on DVE; the y store is deferred to the
            # next pipeline iteration (ACT ring) so it never waits in-queue
            o1 = opool.tile([P, NCH], f32, tag="o", name=f"o1_{tt}_{cb}")
            nc.vector.scalar_tensor_tensor(
                out=o1, in0=ps, scalar=sx[:, 0:1],
                in1=sw_b[:, cb * NCH : (cb + 1) * NCH],
                op0=Alu.mult, op1=Alu.mult,
            )
            o2 = opool.tile([P, NCH], f32, tag="o", name=f"o2_{tt}_{cb}")
            nc.vector.tensor_add(
                out=o2, in0=o1, in1=bb_b[:, cb * NCH : (cb + 1) * NCH]
            )
            return o2

        def store_y(tt, o2s):
            for cb, o2 in enumerate(o2s):
                nc.scalar.dma_start(
                    out=y_ap[tt * P : (tt + 1) * P, cb * NCH : (cb + 1) * NCH],
                    in_=o2,
                )

        # ---- startup: loads interleaved with quant emission; x0 first ----
        x_t = {0: load_x(0)}
        w_t = {0: load_w(0), 1: load_w(1)}
        sx = {}
        qx = {}
        qw = {}
        sx[0], qx[0] = quant_x(0, x_t[0])
        w_t[2] = load_w(2)
        w_t[3] = load_w(3)
        x_t[1] = load_x(1)
        qw[0] = quant_w(0, w_t[0])
        qw[1] = quant_w(1, w_t[1])
        x_t[2] = load_x(2)
        w_t[4] = load_w(4)
        w_t[5] = load_w(5)
        qw[2] = quant_w(2, w_t[2])
        qw[3] = quant_w(3, w_t[3])
        w_t[6] = load_w(6)
        w_t[7] = load_w(7)
        sx[1], qx[1] = quant_x(1, x_t[1])
        sx[2], qx[2] = quant_x(2, x_t[2])
        for wt in range(WPH, WT):
            qw[wt] = quant_w(wt, w_t[wt])

        # scale/bias broadcasts (ACT ring; sw stores were emitted per block)
        nc.scalar.dma_start(
            out=sw_b,
            in_=bass.AP(tensor=sw_dram.tensor, offset=sw_dram.offset, ap=[[0, P], [1, O]]),
        )
        nc.scalar.dma_start(
            out=bb_b,
            in_=bass.AP(tensor=b_ap.tensor, offset=b_ap.offset, ap=[[0, P], [1, O]]),
        )

        # transposes on the (otherwise empty) Sync ring, in readiness order
        qxT = {}
        qxT[0] = transpose_x(0, qx.pop(0))
        for wt in range(WPH):
            transpose_w(wt, qw[wt])
        qxT[1] = transpose_x(1, qx.pop(1))
        qxT[2] = transpose_x(2, qx.pop(2))
        for wt in range(WPH, WT):
            transpose_w(wt, qw[wt])

        # ---- main loop: software pipeline ----
        # iteration tt: y(tt-1) stores, load(tt+2)+quant(tt+2),
        # transpose(tt+1), mm(tt)+epilogue(tt).
        # PE ramp: cb0 groups of tiles 0-2 first (only need qwT half 0).
        o2s = {}
        for tt in range(TT):
            if tt == NPRE:
                # tiles 0..NPRE-1 are complete only once their cb1 groups ran
                for t2 in range(NPRE):
                    store_y(t2, o2s.pop(t2))
            elif tt > NPRE and tt - 1 in o2s:
                store_y(tt - 1, o2s.pop(tt - 1))
            if NPRE <= tt + 2 < TT:
                x_t[tt + 2] = load_x(tt + 2)
                sx[tt + 2], qx[tt + 2] = quant_x(tt + 2, x_t[tt + 2])
            if NPRE <= tt + 1 < TT:
                qxT[tt + 1] = transpose_x(tt + 1, qx.pop(tt + 1))
            if tt < NPRE:
                ps = mm_group(tt, 0, qxT[tt])
                o2s.setdefault(tt, []).append(epilogue(tt, 0, sx[tt], ps))
                if tt == NPRE - 1:
                    for t2 in range(NPRE):
                        ps = mm_group(t2, 1, qxT[t2])
                        o2s[t2].append(epilogue(t2, 1, sx[t2], ps))
            else:
                o2s[tt] = []
                for cb in range(CB):
                    ps = mm_group(tt, cb, qxT[tt])
                    o2s[tt].append(epilogue(tt, cb, sx[tt], ps))
        for tt in sorted(o2s):
            store_y(tt, o2s[tt])
    nc.compile()
    return nc


_cached_nc = None


def _get_nc():
    global _cached_nc
    if _cached_nc is None:
        _cached_nc = build_nc(T_SH, D_IN, O_SH)
    return _cached_nc


def kernel(x: np.ndarray, w: np.ndarray, b: np.ndarray, _trace=False):
    from concourse.bass_utils import run_bass_kernel_spmd

    assert x.shape == (B, S, D_IN) and w.shape == (D_OUT, D_IN) and b.shape == (D_OUT,)
    x2 = np.ascontiguousarray(x.reshape(TOK, D_IN), dtype=np.float32)
    w2 = np.ascontiguousarray(w, dtype=np.float32)
    b2 = np.ascontiguousarray(b, dtype=np.float32)

    in_maps = []
    for core in range(8):
        tg, cg = divmod(core, CH_GROUPS)
        in_maps.append(
            {
                "x": np.ascontiguousarray(x2[tg * T_SH : (tg + 1) * T_SH]),
                "w": np.ascontiguousarray(w2[cg * O_SH : (cg + 1) * O_SH]),
                "b": np.ascontiguousarray(b2[cg * O_SH : (cg + 1) * O_SH]),
            }
        )

    nc = _get_nc()
    res = run_bass_kernel_spmd(nc, in_maps, core_ids=list(range(8)), trace=_trace)

    y = np.empty((TOK, D_OUT), dtype=np.float32)
    for core in range(8):
        tg, cg = divmod(core, CH_GROUPS)
        y[tg * T_SH : (tg + 1) * T_SH, cg * O_SH : (cg + 1) * O_SH] = res.results[
            core
        ]["y"]
    if _trace:
        kernel._last_results = res
    return y.reshape(B, S, D_OUT)


# revision 20
# speedup vs baseline: 1.2916x; 1.2916x over previous
"""Trainium2 Bass kernel for fake-quant (W8A8) linear: y = fq_tok(x) @ fq_ch(w).T + b.

Full shapes: x [4, 2048, 4096] f32, w [4096, 4096] f32, b [4096] f32.
Sharding over 8 cores: 2 token groups x 4 out-channel groups.
Per core: x_sh [4096, 4096], w_sh [1024, 4096], b_sh [1024] -> y_sh [4096, 1024].

Quantized values are integers in [-127, 127], exactly representable in bf16,
so the matmul runs on the PE array in bf16 (full rate) with fp32 PSUM
accumulation - numerically equivalent to the fp32 reference einsum on the
dequantized values.  Scales are applied in the fp32 epilogue.

Design (v6): the host supplies x PRE-TRANSPOSED per 128-token tile
(x[tt, k, t] = x_orig[tt*128+t, k], a pure layout permutation done during
sharding), so the kernel never transposes activations: the quantized tile
IS the matmul's stationary operand layout.  Per-token amax in this layout
is a k-reduction that spans partitions: DVE reduces the 32 k-blocks per
partition (strided inner view), then a small GPSIMD partition_all_reduce
(absmax, [128,128]) folds the 128 partitions and leaves the result
broadcast on all partitions.  Quant chain: DVE tensor_tensor multiply by
the per-token reciprocal row, ACT +MAGIC (f32 rounds), ACT -MAGIC -> bf16.
The PE runs only the weight-phase transposes and the 2048 N=512 bf16
matmuls (216 ns roofline each).

Weights keep the row-major baseline path: DVE amax, round pass 1 on
DVE/ACT (split), pass 2 on ACT, PE-transpose via identity matmuls into
PSUM, copies to SBUF split across DVE/ACT.  qwT is stored in two 512-ch
halves and the PE ramp runs cb0 matmul groups of tiles 0-2 first so
matmuls start once w blocks 0-3 are resident.

DMA transposes were tried and rejected: the ucode DMA_TRANSPOSE blocks its
HWDGE ring ~5us, the scheduler globally serializes every other DMA against
an in-flight transpose, and transposes on both rings at once crash the
device.

Rounding: round-half-to-even via the fp32 magic-constant trick
(v + 1.5*2^23 rounds mantissa to integer; subtract again afterwards),
matching jnp.round.  Clipping to [-128, 127] is a no-op by construction
(|x|/s <= 127 when s = amax/127) so it is skipped.
"""

from contextlib import ExitStack

import numpy as np

import concourse.bass as bass
import concourse.bass_isa as bass_isa
import concourse.mybir as mybir
import concourse.tile as tile
from concourse import bacc
from concourse.masks import make_identity

P = 128
MAGIC = 12582912.0  # 1.5 * 2**23
QMAX = 127.0
EPS = 1e-8

# full problem shapes (hardcoded per harness contract)
B, S, D_IN, D_OUT = 4, 2048, 4096, 4096
TOK = B * S  # 8192
TOK_GROUPS = 2
CH_GROUPS = 4
T_SH = TOK // TOK_GROUPS  # 4096 tokens per core
O_SH = D_OUT // CH_GROUPS  # 1024 channels per core


def build_nc(T, K, O, nch=512):
    """Per-core program: xT[T/128, K, 128] (pre-transposed tiles), w[O,K],
    b[O] -> y[T,O]."""
    f32 = mybir.dt.float32
    bf16 = mybir.dt.bfloat16
    Copy = mybir.ActivationFunctionType.Copy
    Alu = mybir.AluOpType
    AxX = mybir.AxisListType.X

    assert T % P == 0 and K % P == 0 and O % P == 0
    TT, KB, WT = T // P, K // P, O // P
    NCH = min(nch, O)
    CB = O // NCH
    assert CB == 2 and WT == 8, "ramp assumes 2 halves x 4 blocks"
    WPH = WT // CB
    NPRE = 3  # x tiles quantized during the w phase

    nc = bacc.Bacc("TRN2", target_bir_lowering=False, debug=False)
    x_ap = nc.dram_tensor("x", [TT, P, KB, P], f32, kind="ExternalInput").ap()
    w_ap = nc.dram_tensor("w", [O, K], f32, kind="ExternalInput").ap()
    b_ap = nc.dram_tensor("b", [O], f32, kind="ExternalInput").ap()
    y_ap = nc.dram_tensor("y", [T, O], f32, kind="ExternalOutput").ap()

    with tile.TileContext(nc) as tc, ExitStack() as ctx:
        singles = ctx.enter_context(tc.tile_pool(name="singles", bufs=1))
        xtp = ctx.enter_context(tc.tile_pool(name="xtp", bufs=3))
        rnd = ctx.enter_context(tc.tile_pool(name="rnd", bufs=1))
        rnd2 = ctx.enter_context(tc.tile_pool(name="rnd2", bufs=1))
        qpool = ctx.enter_context(tc.tile_pool(name="qpool", bufs=3))
        qwpool = ctx.enter_context(tc.tile_pool(name="qwpool", bufs=2))
        stats = ctx.enter_context(tc.tile_pool(name="stats", bufs=8))
        sxpool = ctx.enter_context(tc.tile_pool(name="sxpool", bufs=5))
        opool = ctx.enter_context(tc.tile_pool(name="opool", bufs=4))
        psum_pool = ctx.enter_context(tc.tile_pool(name="psum", bufs=4, space="PSUM"))
        tpsum = ctx.enter_context(tc.tile_pool(name="tpsum", bufs=3, space="PSUM"))
        dram = ctx.enter_context(tc.tile_pool(name="dram", bufs=1, space="DRAM"))

        qwT = [singles.tile([P, KB, NCH], bf16, name=f"qwT{h}") for h in range(CB)]
        sw_b = singles.tile([P, O], f32)
        bb_b = singles.tile([P, O], f32)
        sw_dram = dram.tile([O, 1], f32)
        ident = singles.tile([P, P], bf16)
        make_identity(nc, ident)

        TG = min(8, KB)

        def pe_transpose(q_sbuf, dst, tag, copy_eng):
            # q_sbuf [P, K] bf16 -> dst[f, k, c] = q_sbuf[c, k*128+f]
            for g in range(KB // TG):
                tp = tpsum.tile([P, TG, P], bf16, tag="tp", name=f"tp_{tag}_{g}")
                for j in range(TG):
                    kb = g * TG + j
                    nc.tensor.transpose(
                        tp[:, j, :], q_sbuf[:, kb * P : (kb + 1) * P], ident
                    )
                dslice = dst[:, g * TG : (g + 1) * TG, :]
                if copy_eng == "act":
                    nc.scalar.activation(out=dslice, in_=tp, func=Copy)
                else:
                    nc.vector.tensor_copy(out=dslice, in_=tp)

        # ---- weight-phase stages (row-major w, baseline path) ----
        def load_w(wt):
            w_t = xtp.tile([P, K], f32, tag="big", name=f"w_{wt}")
            nc.sync.dma_start(out=w_t, in_=w_ap[wt * P : (wt + 1) * P, :])
            return w_t

        def quant_w(wt, w_t):
            sw = stats.tile([P, 1], f32, tag="st", name=f"sw_{wt}")
            amax = stats.tile([P, 1], f32, tag="st", name=f"am_{wt}")
            nc.vector.reduce_max(
                out=amax, in_=w_t, axis=AxX, apply_absolute_value=True
            )
            nc.vector.tensor_scalar(
                out=sw, in0=amax, scalar1=1.0 / QMAX, scalar2=EPS,
                op0=Alu.mult, op1=Alu.max,
            )
            r_t = stats.tile([P, 1], f32, tag="st", name=f"rw_{wt}")
            nc.vector.reciprocal(out=r_t, in_=sw)
            t_t = rnd.tile([P, K], f32, tag="rnd", name=f"tw_{wt}")
            if wt >= WPH:
                # late blocks round pass 1 on DVE to unclog ACT
                nc.vector.tensor_scalar(
                    out=t_t, in0=w_t, scalar1=r_t[:, 0:1], scalar2=MAGIC,
                    op0=Alu.mult, op1=Alu.add,
                )
            else:
                nc.scalar.activation(
                    out=t_t, in_=w_t, func=Copy, bias=MAGIC, scale=r_t[:, 0:1]
                )
            qw = qwpool.tile([P, K], bf16, tag="qw", name=f"qw_{wt}")
            nc.scalar.activation(out=qw, in_=t_t, func=Copy, bias=-MAGIC, scale=1.0)
            nc.sync.dma_start(out=sw_dram[wt * P : (wt + 1) * P, :], in_=sw)
            return qw

        def transpose_w(wt, qw):
            h, c = divmod(wt, WPH)
            pe_transpose(
                qw, qwT[h][:, :, c * P : (c + 1) * P], f"w{wt}",
                copy_eng=("dve" if h == 0 else "act"),
            )

        # ---- x stages (pre-transposed tiles; no on-device transpose) ----
        def load_x(tt):
            x_t = xtp.tile([P, KB, P], f32, tag="big", name=f"x_{tt}")
            nc.sync.dma_start(out=x_t, in_=x_ap[tt, :, :, :])
            return x_t

        def quant_x(tt, x_t):
            # per-token amax: DVE folds the 32 k-blocks per partition
            # (strided inner view [p, t, kb]), GPSIMD folds the partitions
            pmax = stats.tile([P, P], f32, tag="pm", name=f"pm_{tt}")
            x_sw = bass.AP(
                tensor=x_t.tensor, offset=x_t.offset,
                ap=[list(x_t.ap[0]), [1, P], [P, KB]],
            )
            nc.vector.reduce_max(
                out=pmax, in_=x_sw, axis=AxX, apply_absolute_value=True
            )
            amax = stats.tile([P, P], f32, tag="pm", name=f"am_{tt}")
            nc.gpsimd.partition_all_reduce(
                amax, pmax, P, bass_isa.ReduceOp.absmax
            )
            sx = sxpool.tile([P, P], f32, tag="sx", name=f"sx_{tt}")
            nc.vector.tensor_scalar(
                out=sx, in0=amax, scalar1=1.0 / QMAX, scalar2=EPS,
                op0=Alu.mult, op1=Alu.max,
            )
            r_t = stats.tile([P, P], f32, tag="pm", name=f"rx_{tt}")
            nc.vector.reciprocal(out=r_t, in_=sx)
            # the epilogue needs scale[t] on partition t as a [P,1] pointer:
            # roundtrip row 0 of sx through DRAM (DRAM[t] -> partition t)
            sxd_d = dram.tile([P], f32, tag="sxd", name=f"sxd_{tt}")
            nc.sync.dma_start(out=sxd_d, in_=sx[0:1, :])
            sxd = sxpool.tile([P, 1], f32, tag="sxc", name=f"sxc_{tt}")
            nc.sync.dma_start(
                out=sxd,
                in_=bass.AP(tensor=sxd_d.tensor, offset=sxd_d.offset, ap=[[1, P], [1, 1]]),
            )
            # round: DVE multiply by the recip row (broadcast over kb via a
            # zero-stride mid dim), ACT +MAGIC (f32 rounds), ACT -MAGIC->bf16
            t1 = rnd.tile([P, KB, P], f32, tag="rnd", name=f"t1_{tt}")
            r_bc = bass.AP(
                tensor=r_t.tensor, offset=r_t.offset,
                ap=[list(r_t.ap[0]), [0, KB], [1, P]],
            )
            nc.vector.tensor_tensor(out=t1, in0=x_t, in1=r_bc, op=Alu.mult)
            t2 = rnd2.tile([P, KB, P], f32, tag="rnd2", name=f"t2_{tt}")
            nc.scalar.activation(out=t2, in_=t1, func=Copy, bias=MAGIC, scale=1.0)
            qxT = qpool.tile([P, KB, P], bf16, tag="q", name=f"qx_{tt}")
            nc.scalar.activation(out=qxT, in_=t2, func=Copy, bias=-MAGIC, scale=1.0)
            return sxd, qxT

        def mm_group(tt, cb, qxT):
            ps = psum_pool.tile([P, NCH], f32, tag="psum", name=f"ps_{tt}_{cb}")
            for k in range(KB):
                nc.tensor.matmul(
                    ps,
                    qxT[:, k, :],
                    qwT[cb][:, k, :],
                    start=(k == 0),
                    stop=(k == KB - 1),
                )
            return ps

        def epilogue(tt, cb, sxd, ps):
            # psum[t, ch] * sx[t] * sw[ch] + b[ch]; sxd is the [P,1] scale
            # column (DRAM roundtrip put scale[t] on partition t)
            o1 = opool.tile([P, NCH], f32, tag="o", name=f"o1_{tt}_{cb}")
            nc.vector.scalar_tensor_tensor(
                out=o1, in0=ps, scalar=sxd[:, 0:1],
                in1=sw_b[:, cb * NCH : (cb + 1) * NCH],
                op0=Alu.mult, op1=Alu.mult,
            )
            o2 = opool.tile([P, NCH], f32, tag="o", name=f"o2_{tt}_{cb}")
            nc.vector.tensor_add(
                out=o2, in0=o1, in1=bb_b[:, cb * NCH : (cb + 1) * NCH]
            )
            return o2

        def store_y(tt, o2s):
            for cb, o2 in enumerate(o2s):
                nc.sync.dma_start(
                    out=y_ap[tt * P : (tt + 1) * P, cb * NCH : (cb + 1) * NCH],
                    in_=o2,
                )

        # ---- startup: loads up-front; x0 quant leads; w blocks follow ----
        x_t = {0: load_x(0)}
        w_t = {wt: load_w(wt) for wt in range(WPH)}
        x_t[1] = load_x(1)
        x_t[2] = load_x(2)
        for wt in range(WPH, WT):
            w_t[wt] = load_w(wt)

        sx = {}
        qxT = {}
        qw = {}
        sx[0], qxT[0] = quant_x(0, x_t.pop(0))
        for wt in range(WPH):
            qw[wt] = quant_w(wt, w_t[wt])
        sx[1], qxT[1] = quant_x(1, x_t.pop(1))
        sx[2], qxT[2] = quant_x(2, x_t.pop(2))
        for wt in range(WPH, WT):
            qw[wt] = quant_w(wt, w_t[wt])

        nc.sync.dma_start(
            out=sw_b,
            in_=bass.AP(tensor=sw_dram.tensor, offset=sw_dram.offset, ap=[[0, P], [1, O]]),
        )
        nc.sync.dma_start(
            out=bb_b,
            in_=bass.AP(tensor=b_ap.tensor, offset=b_ap.offset, ap=[[0, P], [1, O]]),
        )

        # PE: w transposes (half 0 first), then the ramp + steady matmuls
        for wt in range(WPH):
            transpose_w(wt, qw[wt])

        def store_one(tt, cb, o2):
            nc.sync.dma_start(
                out=y_ap[tt * P : (tt + 1) * P, cb * NCH : (cb + 1) * NCH],
                in_=o2,
            )

        o2s = {}
        emitted_late_w = False
        for tt in range(TT):
            if tt - 1 in o2s:
                store_y(tt - 1, o2s.pop(tt - 1))
            if NPRE <= tt + 2 < TT:
                x_t[tt + 2] = load_x(tt + 2)
            if NPRE <= tt + 1 < TT:
                sx[tt + 1], qxT[tt + 1] = quant_x(tt + 1, x_t.pop(tt + 1))
            if tt < NPRE:
                # ramp: store each half as soon as its epilogue runs (short
                # o2 lifetimes keep opool small)
                ps = mm_group(tt, 0, qxT[tt])
                store_one(tt, 0, epilogue(tt, 0, sx[tt], ps))
                # late w transposes interleave after the first cb0 group
                if not emitted_late_w:
                    for wt in range(WPH, WT):
                        transpose_w(wt, qw[wt])
                    emitted_late_w = True
                if tt == NPRE - 1:
                    for t2 in range(NPRE):
                        ps = mm_group(t2, 1, qxT[t2])
                        store_one(t2, 1, epilogue(t2, 1, sx[t2], ps))
            else:
                o2s[tt] = []
                for cb in range(CB):
                    ps = mm_group(tt, cb, qxT[tt])
                    o2s[tt].append(epilogue(tt, cb, sx[tt], ps))
        for tt in sorted(o2s):
            store_y(tt, o2s[tt])
    nc.compile()
    return nc


_cached_nc = None


def _get_nc():
    global _cached_nc
    if _cached_nc is None:
        _cached_nc = build_nc(T_SH, D_IN, O_SH)
    return _cached_nc


def kernel(x: np.ndarray, w: np.ndarray, b: np.ndarray, _trace=False):
    from concourse.bass_utils import run_bass_kernel_spmd

    assert x.shape == (B, S, D_IN) and w.shape == (D_OUT, D_IN) and b.shape == (D_OUT,)
    x2 = np.ascontiguousarray(x.reshape(TOK, D_IN), dtype=np.float32)
    w2 = np.ascontiguousarray(w, dtype=np.float32)
    b2 = np.ascontiguousarray(b, dtype=np.float32)

    in_maps = []
    for core in range(8):
        tg, cg = divmod(core, CH_GROUPS)
        xs = x2[tg * T_SH : (tg + 1) * T_SH]
        # pre-transpose per 128-token tile into the SBUF layout
        # xt[tt, p, kb, t] = x[tt*128+t, kb*128+p]
        TT = T_SH // P
        KB = D_IN // P
        xt = np.ascontiguousarray(
            xs.reshape(TT, P, KB, P).transpose(0, 3, 2, 1)
        )
        in_maps.append(
            {
                "x": xt,
                "w": np.ascontiguousarray(w2[cg * O_SH : (cg + 1) * O_SH]),
                "b": np.ascontiguousarray(b2[cg * O_SH : (cg + 1) * O_SH]),
            }
        )

    nc = _get_nc()
    res = run_bass_kernel_spmd(nc, in_maps, core_ids=list(range(8)), trace=_trace)

    y = np.empty((TOK, D_OUT), dtype=np.float32)
    for core in range(8):
        tg, cg = divmod(core, CH_GROUPS)
        y[tg * T_SH : (tg + 1) * T_SH, cg * O_SH : (cg + 1) * O_SH] = res.results[
            core
        ]["y"]
    if _trace:
        kernel._last_results = res
    return y.reshape(B, S, D_OUT)
